# Initial kernel scaffold
#
"""BFLinear (block-floating-point quantized linear) Trainium2 kernel.

Computes: out = bf_quant(bf_quant(x) @ bf_quant(W).T + 2*b)
where bf_quant quantizes groups of 32 along the last axis to a shared
power-of-two exponent with 8 mantissa bits (values = int8 * 2^(e-7)).

Distribution over 8 NeuronCores:
  - batch dim of x sharded 8 ways (1024 rows/core)
  - W quantization split by output rows (512 rows/core), then AllGather of
    the quantized bf16 weights (values are exact in bf16: |int|<=128 * 2^k)
  - matmul runs in bf16 (exact products, fp32 PSUM accumulation)
  - bias (x2) is pre-written into PSUM by the scalar engine, so the matmul
    accumulates on top of it exactly in fp32

Quantization math (all exact, matching jnp semantics):
  m     = max |x| over each group of 32          (DVE abs-max reduce)
  scale = 2^(floor(log2 m) - 7)                  (exponent-field bit math)
  inv   = 1/scale                                (bit math, exact)
  r     = rne_round(x * inv)  via +C trick with C = 1.5*2^23
  q     = clip(r, -128, 127) * scale
"""

import numpy as np

# full-problem dimensions (hardcoded per harness contract)
B_FULL = 8192
IN_FULL = 4096
OUT_FULL = 4096
NCORES = 8

P = 128
SZ = 32
NBLK = 512  # matmul moving free dim / output column block
C_RND = float(3 * 2**22)  # 1.5*2^23: v+C stays in [2^23, 2^24) -> RNE to ints


def build_nc(b_sh=B_FULL // NCORES, in_dim=IN_FULL, out_dim=OUT_FULL,
             ncores=NCORES, n_half=2, for_timeline=False):
    """Build the SPMD Bass program (identical on every core; data differs)."""
    import concourse.bass as bass
    import concourse.mybir as mybir
    import concourse.tile as tile
    from concourse import bacc

    F32 = mybir.dt.float32
    BF16 = mybir.dt.bfloat16
    I32 = mybir.dt.int32
    ALU = mybir.AluOpType
    AX = mybir.AxisListType
    AF = mybir.ActivationFunctionType

    w_sl = out_dim // ncores          # W rows quantized on this core
    k_chunks = in_dim // P            # 128-wide contraction chunks
    o_blks = out_dim // NBLK
    b_half = b_sh // n_half
    assert b_half % P == 0 and in_dim % SZ == 0 and out_dim % NBLK == 0

    nc = bacc.Bacc("TRN2", target_bir_lowering=False, debug=False,
                   num_devices=ncores)

    x_sh = nc.dram_tensor("x_sh", [b_sh, in_dim], F32, kind="ExternalInput")
    w_sl_t = nc.dram_tensor("w_sl", [w_sl, in_dim], F32, kind="ExternalInput")
    b_rep_in = nc.dram_tensor("b_rep", [P, out_dim], F32, kind="ExternalInput")
    out_sh = nc.dram_tensor("out_sh", [b_sh, out_dim], F32, kind="ExternalOutput")

    wq_loc = nc.dram_tensor("wq_loc", [w_sl, in_dim], BF16)
    wq_ag = nc.dram_tensor("wq_ag", [out_dim, in_dim], BF16, addr_space="Shared")
    xq_dram = [nc.dram_tensor(f"xq_h{h}", [b_half, in_dim], BF16)
               for h in range(n_half)]

    with tile.TileContext(nc) as tc:
        from contextlib import ExitStack
        with ExitStack() as ctx:
            qpool = ctx.enter_context(tc.tile_pool(name="qpool", bufs=2))
            spool = ctx.enter_context(tc.tile_pool(name="spool", bufs=2))
            xqt_pool = ctx.enter_context(tc.tile_pool(name="xqt", bufs=2))
            wqt_pool = ctx.enter_context(tc.tile_pool(name="wqt", bufs=6))
            psum_pool = ctx.enter_context(
                tc.tile_pool(name="psum", bufs=8, space="PSUM"))
            opool = ctx.enter_context(tc.tile_pool(name="opool", bufs=3))
            ospool = ctx.enter_context(tc.tile_pool(name="ospool", bufs=3))
            bpool = ctx.enter_context(tc.tile_pool(name="bpool", bufs=1))

            def quant_tile(xt, rows, width, out_dtype, qpool_, spool_, tagp):
                """Quantize an SBUF-resident [rows, width] f32 tile in place
                (xt is clobbered); returns (q_tile, rows) with dtype out_dtype."""
                g = width // SZ
                x3 = xt.rearrange("p (g s) -> p g s", s=SZ)
                m = spool_.tile([rows, g], F32, tag=f"{tagp}_m")
                nc.vector.tensor_reduce(m[:], x3, axis=AX.X, op=ALU.max,
                                        apply_absolute_value=True)
                # (walrus rejects mixing bitwise and arith ops in one
                #  tensor_scalar, so exponent math is split into 1-op insts)
                scale = spool_.tile([rows, g], F32, tag=f"{tagp}_scale")
                # scale_bits = (m_bits & 0x7F800000) - (7 << 23)
                nc.vector.tensor_scalar(
                    scale[:].bitcast(I32), m[:].bitcast(I32),
                    0x7F800000, None, op0=ALU.bitwise_and)
                nc.vector.tensor_scalar(
                    scale[:].bitcast(I32), scale[:].bitcast(I32),
                    7 << 23, None, op0=ALU.subtract)
                inv = spool_.tile([rows, g], F32, tag=f"{tagp}_inv")
                # inv_bits = (254<<23) - scale_bits = (scale_bits ^ -1) + ((254<<23)+1)
                nc.vector.tensor_scalar(
                    inv[:].bitcast(I32), scale[:].bitcast(I32),
                    -1, None, op0=ALU.bitwise_xor)
                nc.vector.tensor_scalar(
                    inv[:].bitcast(I32), inv[:].bitcast(I32),
                    (254 << 23) + 1, None, op0=ALU.add)
                # v = x * inv (exact power-of-two scaling)
                nc.vector.tensor_tensor(
                    x3, x3, inv[:, :, None].to_broadcast([rows, g, SZ]), ALU.mult)
                # pre-round clamp to (-128.5, 127.5) window; equivalent to the
                # reference's post-round clip because round() is monotonic:
                # round(clip(v, -128.25, 127.25)) == clip(round(v), -128, 127)
                nc.vector.tensor_scalar(
                    xt, xt, -128.25, 127.25, op0=ALU.max, op1=ALU.min)
                # r = w + C rounds to nearest-even integer (+C domain); runs on
                # the otherwise-idle scalar engine, in place
                nc.scalar.activation(xt, xt, AF.Copy, bias=C_RND, scale=1.0)
                # q = (r - C) * scale, fused subtract+scale, exact in bf16
                q = qpool_.tile([rows, width], out_dtype, tag=f"{tagp}_q")
                nc.vector.scalar_tensor_tensor(
                    q[:].rearrange("p (g s) -> p g s", s=SZ),
                    xt.rearrange("p (g s) -> p g s", s=SZ),
                    C_RND,
                    scale[:, :, None].to_broadcast([rows, g, SZ]),
                    op0=ALU.subtract, op1=ALU.mult)
                return q

            # ---- W slice quantize -> wq_loc --------------------------------
            wrow = 0
            while wrow < w_sl:
                rows = min(P, w_sl - wrow)
                wt = qpool.tile([rows, in_dim], F32, tag="q_x")
                nc.sync.dma_start(wt[:], w_sl_t.ap()[wrow:wrow + rows, :])
                q = quant_tile(wt[:], rows, in_dim, BF16, qpool, spool, "q")
                nc.gpsimd.dma_start(wq_loc.ap()[wrow:wrow + rows, :], q[:])
                wrow += rows

            # ---- AllGather quantized weights (runs on TOPSP/SDMA) ----------
            if for_timeline:
                # single-core TimelineSim variant: substitute a local DMA so
                # deps exist; timing of the real AG (~30us, off-engine) differs
                nc.sync.dma_start(wq_ag.ap()[0:w_sl, :], wq_loc.ap())
            else:
                nc.gpsimd.collective_compute(
                    "AllGather", ALU.bypass,
                    replica_groups=[list(range(ncores))],
                    ins=[wq_loc.ap().opt()], outs=[wq_ag.ap().opt()])

            # ---- bias (replicated [P, out_dim] on host) --------------------
            b_rep = bpool.tile([P, out_dim], F32, tag="b_rep")
            nc.sync.dma_start(b_rep[:], b_rep_in.ap())

            for h in range(n_half):
                # ---- x half quantize -> xq_dram[h] -------------------------
                for rt in range(b_half // P):
                    xt = qpool.tile([P, in_dim], F32, tag="q_x")
                    nc.sync.dma_start(
                        xt[:], x_sh.ap()[h * b_half + rt * P:
                                         h * b_half + (rt + 1) * P, :])
                    q = quant_tile(xt[:], P, in_dim, BF16, qpool, spool, "q")
                    nc.gpsimd.dma_start(
                        xq_dram[h].ap()[rt * P:(rt + 1) * P, :], q[:])

                # ---- transposed load: xqt[:, k, :] = xq[:, kchunk].T -------
                xqt = xqt_pool.tile([P, k_chunks, b_half], BF16, tag="xqt")
                for k in range(k_chunks):
                    nc.sync.dma_start_transpose(
                        xqt[:, k, :], xq_dram[h].ap()[:, k * P:(k + 1) * P])

                # ---- main matmul + output quant ----------------------------
                # process output-column blocks in pairs: one [P, 2*NBLK]
                # transposed weight load feeds 2 matmuls per lhsT (halves the
                # DMA-issue count and shares the PE stationary load)
                nbb = b_half // P
                pair = 2 if (o_blks % 2 == 0 and nbb * 2 <= 8) else 1
                obw = NBLK * pair
                for obp in range(o_blks // pair):
                    psums = []
                    for bb in range(nbb):
                        row = []
                        for j in range(pair):
                            pt = psum_pool.tile([P, NBLK], F32, tag="psum")
                            # pre-load 2*b into PSUM (exact fp32)
                            col = obp * obw + j * NBLK
                            nc.scalar.activation(
                                pt[:], b_rep[:, col:col + NBLK],
                                AF.Copy, scale=2.0)
                            row.append(pt)
                        psums.append(row)
                    for k in range(k_chunks):
                        wqt = wqt_pool.tile([P, obw], BF16, tag="wqt")
                        nc.sync.dma_start_transpose(
                            wqt[:], wq_ag.ap()[obp * obw:(obp + 1) * obw,
                                               k * P:(k + 1) * P])
                        for bb in range(nbb):
                            for j in range(pair):
                                nc.tensor.matmul(
                                    psums[bb][j][:],
                                    lhsT=xqt[:, k, bb * P:(bb + 1) * P],
                                    rhs=wqt[:, j * NBLK:(j + 1) * NBLK],
                                    start=False, stop=(k == k_chunks - 1),
                                    skip_group_check=True)
                    for bb in range(nbb):
                        s = opool.tile([P, obw], F32, tag="o_s")
                        for j in range(pair):
                            nc.scalar.copy(s[:, j * NBLK:(j + 1) * NBLK],
                                           psums[bb][j][:])
                        oq = quant_tile(s[:], P, obw, F32, opool, ospool, "o")
                        nc.gpsimd.dma_start(
                            out_sh.ap()[h * b_half + bb * P:
                                        h * b_half + (bb + 1) * P,
                                        obp * obw:(obp + 1) * obw], oq[:])

    nc.compile()
    return nc


_NC_CACHE = {}


def _get_nc(key=(B_FULL // NCORES, IN_FULL, OUT_FULL, NCORES, 2)):
    if key not in _NC_CACHE:
        _NC_CACHE[key] = build_nc(*key)
    return _NC_CACHE[key]


def make_in_maps(x, W, b, ncores=NCORES):
    b_sh = x.shape[0] // ncores
    w_sl = W.shape[0] // ncores
    out_dim = W.shape[0]
    b_rep = np.ascontiguousarray(
        np.broadcast_to(np.asarray(b, np.float32).reshape(1, out_dim),
                        (P, out_dim)))
    return [
        {
            "x_sh": np.ascontiguousarray(x[c * b_sh:(c + 1) * b_sh]),
            "w_sl": np.ascontiguousarray(W[c * w_sl:(c + 1) * w_sl]),
            "b_rep": b_rep,
        }
        for c in range(ncores)
    ]


def kernel(x, W, b):
    from concourse.bass_utils import run_bass_kernel_spmd

    x = np.asarray(x, np.float32)
    W = np.asarray(W, np.float32)
    b = np.asarray(b, np.float32)
    nc = _get_nc()
    in_maps = make_in_maps(x, W, b)
    res = run_bass_kernel_spmd(nc, in_maps, core_ids=list(range(NCORES)))
    return np.concatenate([res.results[c]["out_sh"] for c in range(NCORES)],
                          axis=0)



# revision 4
# speedup vs baseline: 2.4370x; 2.4370x over previous
"""BFLinear (block-floating-point quantized linear) Trainium2 kernel.

Computes: out = bf_quant(bf_quant(x) @ bf_quant(W).T + 2*b)
where bf_quant quantizes groups of 32 along the last axis to a shared
power-of-two exponent with 8 mantissa bits (values = int8 * 2^(e-7)).

Distribution over 8 NeuronCores:
  - batch dim of x sharded 8 ways (1024 rows/core)
  - W quantization split by output rows (512 rows/core); the quantized
    slab is transposed to [in, out] layout on the PE array (identity
    matmuls) while the PE is otherwise idle, then AllGathered as bf16 in
    two 256-column halves so matmuls against the first half can start
    while the second AllGather is still in flight
  - x is quantized on-chip and PE-transposed into a resident SBUF
    [in, batch] buffer - no DRAM round trip
  - matmul runs in bf16 (quantized values are exact in bf16), fp32 PSUM
    accumulation, k-innermost per PSUM bank; weight slabs stream from the
    gathered buffer as contiguous [128, 32, 256] tiles (no transposed DMA
    in the hot loop)
  - bias (x2) is added during the PSUM drain on the vector engine (fp32)
    - NOT pre-loaded into PSUM, which loses it to the has_written
    overwrite on fresh banks - then the output is quantized and stored as
    bf16 (exact for quantized values); the host casts back to fp32

Quantization math (matching jnp semantics):
  m     = max |x| over each group of 32          (DVE abs-max reduce)
  scale = 2^(floor(log2 m) - 7)                  (exponent-field bit math)
  inv   = 1/scale                                (bit math, exact)
  v     = clamp(x*inv, -128.25, 127.25)          (== post-round clip, round
                                                  is monotonic)
  r     = rne_round(v) via +C trick, C = 1.5*2^23 (scalar engine)
  q     = (r - C) * scale                        (exact in bf16)
"""

import numpy as np

# full-problem dimensions (hardcoded per harness contract)
B_FULL = 8192
IN_FULL = 4096
OUT_FULL = 4096
NCORES = 8

P = 128
SZ = 32
C_RND = float(3 * 2**22)  # 1.5*2^23: v+C stays in [2^23, 2^24) -> RNE to ints


def build_nc(b_sh=B_FULL // NCORES, in_dim=IN_FULL, out_dim=OUT_FULL,
             ncores=NCORES, for_timeline=False):
    """Build the SPMD Bass program (identical on every core; data differs)."""
    import concourse.mybir as mybir
    import concourse.tile as tile
    from concourse import bacc, masks

    F32 = mybir.dt.float32
    BF16 = mybir.dt.bfloat16
    I32 = mybir.dt.int32
    ALU = mybir.AluOpType
    AX = mybir.AxisListType
    AF = mybir.ActivationFunctionType

    w_sl = out_dim // ncores      # W rows quantized on this core (512)
    k_chunks = in_dim // P        # 128-wide contraction chunks (32)
    nbb = b_sh // P               # batch row blocks (8)
    half = w_sl // 2              # o-columns per AG half (256)
    g_half = half // SZ           # quant groups per drain tile (8)

    nc = bacc.Bacc("TRN2", target_bir_lowering=False, debug=False,
                   num_devices=ncores)

    x_sh = nc.dram_tensor("x_sh", [b_sh, in_dim], F32, kind="ExternalInput")
    w_sl_t = nc.dram_tensor("w_sl", [w_sl, in_dim], F32, kind="ExternalInput")
    b2_rep = nc.dram_tensor("b2_rep", [P, out_dim], F32, kind="ExternalInput")
    out_sh = nc.dram_tensor("out_sh", [b_sh, out_dim], BF16,
                            kind="ExternalOutput")

    wqT_lo = nc.dram_tensor("wqT_lo", [in_dim, half], BF16)
    wqT_hi = nc.dram_tensor("wqT_hi", [in_dim, half], BF16)
    ag_lo = nc.dram_tensor("ag_lo", [ncores * in_dim, half], BF16,
                           addr_space="Shared")
    ag_hi = nc.dram_tensor("ag_hi", [ncores * in_dim, half], BF16,
                           addr_space="Shared")

    with tile.TileContext(nc) as tc:
        from contextlib import ExitStack
        with ExitStack() as ctx:
            xpool = ctx.enter_context(tc.tile_pool(name="xpool", bufs=2))
            qpool = ctx.enter_context(tc.tile_pool(name="qpool", bufs=2))
            spool = ctx.enter_context(tc.tile_pool(name="spool", bufs=2))
            wpool = ctx.enter_context(tc.tile_pool(name="wpool", bufs=2))
            bpool = ctx.enter_context(tc.tile_pool(name="bpool", bufs=2))
            dpool = ctx.enter_context(tc.tile_pool(name="dpool", bufs=3))
            opool = ctx.enter_context(tc.tile_pool(name="opool", bufs=3))
            dsp = ctx.enter_context(tc.tile_pool(name="dsp", bufs=3))
            pmm = ctx.enter_context(
                tc.tile_pool(name="pmm", bufs=5, space="PSUM"))
            ptp = ctx.enter_context(
                tc.tile_pool(name="ptp", bufs=3, space="PSUM"))

            ident, _ = tc.tile([P, P], BF16, name="ident")
            masks.make_identity(nc, ident[:])

            # resident transposed operands
            xqT, _ = tc.tile([P, k_chunks, b_sh], BF16, name="xqT")
            wqt, _ = tc.tile([P, k_chunks, w_sl], BF16, name="wqt")

            def quant(t, width, q_out, sp, tagp):
                """Quantize an SBUF-resident [P, width] f32 tile (t is
                clobbered) into q_out (any dtype; bf16 is exact)."""
                g = width // SZ
                t3 = t.rearrange("p (g s) -> p g s", s=SZ)
                m = sp.tile([P, g], F32, tag=f"{tagp}_m")
                nc.vector.tensor_reduce(m[:], t3, axis=AX.X, op=ALU.max,
                                        apply_absolute_value=True)
                scale = sp.tile([P, g], F32, tag=f"{tagp}_scale")
                # scale_bits = (m_bits & 0x7F800000) - (7 << 23)
                nc.vector.tensor_scalar(
                    scale[:].bitcast(I32), m[:].bitcast(I32),
                    0x7F800000, None, op0=ALU.bitwise_and)
                nc.vector.tensor_scalar(
                    scale[:].bitcast(I32), scale[:].bitcast(I32),
                    7 << 23, None, op0=ALU.subtract)
                inv = sp.tile([P, g], F32, tag=f"{tagp}_inv")
                # inv_bits = (254<<23) - scale_bits
                #          = (scale_bits ^ -1) + ((254<<23)+1)
                nc.vector.tensor_scalar(
                    inv[:].bitcast(I32), scale[:].bitcast(I32),
                    -1, None, op0=ALU.bitwise_xor)
                nc.vector.tensor_scalar(
                    inv[:].bitcast(I32), inv[:].bitcast(I32),
                    (254 << 23) + 1, None, op0=ALU.add)
                # v = x * inv (exact power-of-two scaling)
                nc.vector.tensor_tensor(
                    t3, t3, inv[:, :, None].to_broadcast([P, g, SZ]),
                    ALU.mult)
                # pre-round clamp; == post-round clip (round is monotonic)
                nc.vector.tensor_scalar(
                    t, t, -128.25, 127.25, op0=ALU.max, op1=ALU.min)
                # +C forces RNE-to-integer on the scalar engine
                nc.scalar.activation(t, t, AF.Copy, bias=C_RND, scale=1.0)
                # q = (r - C) * scale, fused subtract+scale
                nc.vector.scalar_tensor_tensor(
                    q_out.rearrange("p (g s) -> p g s", s=SZ),
                    t3, C_RND,
                    scale[:, :, None].to_broadcast([P, g, SZ]),
                    op0=ALU.subtract, op1=ALU.mult)

            def pe_transpose_into(src_bf16, dest, col_base):
                """PE-transpose [P, in_dim] bf16 src into dest[:, k, col_base:
                col_base+P] for every k chunk (dest is a [P, k_chunks, *]
                SBUF tile)."""
                for k in range(k_chunks):
                    pst = ptp.tile([P, P], BF16, tag="pst",
                                   padded_shape=[P, 1024])
                    nc.tensor.matmul(pst[:], lhsT=src_bf16[:, k * P:(k + 1) * P],
                                     rhs=ident[:], is_transpose=True,
                                     skip_group_check=True)
                    nc.scalar.copy(dest[:, k, col_base:col_base + P], pst[:])

            # ---- W slice: quantize + PE-transpose into wqt ------------------
            for r in range(w_sl // P):
                wt = xpool.tile([P, in_dim], F32, tag="ld")
                nc.sync.dma_start(wt[:], w_sl_t.ap()[r * P:(r + 1) * P, :])
                wq = qpool.tile([P, in_dim], BF16, tag="q")
                quant(wt[:], in_dim, wq[:], spool, "q")
                pe_transpose_into(wq[:], wqt, r * P)

            # ---- store transposed slab halves + AllGather them -------------
            nc.gpsimd.dma_start(
                wqT_lo.ap().rearrange("(k p) o -> p k o", p=P),
                wqt[:, :, 0:half])
            nc.gpsimd.dma_start(
                wqT_hi.ap().rearrange("(k p) o -> p k o", p=P),
                wqt[:, :, half:w_sl])
            if for_timeline:
                nc.sync.dma_start(ag_lo.ap()[0:in_dim, :], wqT_lo.ap())
                nc.sync.dma_start(ag_hi.ap()[0:in_dim, :], wqT_hi.ap())
            else:
                nc.gpsimd.collective_compute(
                    "AllGather", ALU.bypass,
                    replica_groups=[list(range(ncores))],
                    ins=[wqT_lo.ap().opt()], outs=[ag_lo.ap().opt()])
                nc.gpsimd.collective_compute(
                    "AllGather", ALU.bypass,
                    replica_groups=[list(range(ncores))],
                    ins=[wqT_hi.ap().opt()], outs=[ag_hi.ap().opt()])

            # ---- x: quantize + PE-transpose into resident xqT --------------
            for bb in range(nbb):
                xt = xpool.tile([P, in_dim], F32, tag="ld")
                nc.sync.dma_start(xt[:], x_sh.ap()[bb * P:(bb + 1) * P, :])
                xq = qpool.tile([P, in_dim], BF16, tag="q")
                quant(xt[:], in_dim, xq[:], spool, "q")
                pe_transpose_into(xq[:], xqT, bb * P)

            # ---- matmul waves: 8 lo-half units, then 8 hi-half units -------
            for h, ag in ((0, ag_lo), (1, ag_hi)):
                for j in range(ncores):
                    col = j * w_sl + h * half
                    slab = wpool.tile([P, k_chunks, half], BF16, tag="slab")
                    nc.scalar.dma_start(
                        slab[:],
                        ag.ap()[j * in_dim:(j + 1) * in_dim, :]
                        .rearrange("(k p) o -> p k o", p=P))
                    b2s = bpool.tile([P, half], F32, tag="b2s")
                    nc.scalar.dma_start(b2s[:], b2_rep.ap()[:, col:col + half])
                    for bb in range(nbb):
                        ps = pmm.tile([P, half], F32, tag="ps",
                                      padded_shape=[P, 512])
                        for k in range(k_chunks):
                            nc.tensor.matmul(
                                ps[:],
                                lhsT=xqT[:, k, bb * P:(bb + 1) * P],
                                rhs=slab[:, k, :],
                                start=(k == 0), stop=(k == k_chunks - 1),
                                skip_group_check=True)
                        s = dpool.tile([P, half], F32, tag="s")
                        # bias (x2, baked into b2_rep host-side) added during
                        # the PSUM drain - exact fp32
                        nc.vector.tensor_tensor(s[:], ps[:], b2s[:], ALU.add)
                        oq = opool.tile([P, half], BF16, tag="oq")
                        quant(s[:], half, oq[:], dsp, "d")
                        nc.gpsimd.dma_start(
                            out_sh.ap()[bb * P:(bb + 1) * P, col:col + half],
                            oq[:])

    nc.compile()
    return nc


_NC_CACHE = {}


def _get_nc(key=(B_FULL // NCORES, IN_FULL, OUT_FULL, NCORES)):
    if key not in _NC_CACHE:
        _NC_CACHE[key] = build_nc(*key)
    return _NC_CACHE[key]


def make_in_maps(x, W, b, ncores=NCORES):
    b_sh = x.shape[0] // ncores
    w_sl = W.shape[0] // ncores
    out_dim = W.shape[0]
    b2_rep = np.ascontiguousarray(
        np.broadcast_to((2.0 * np.asarray(b, np.float32)).reshape(1, out_dim),
                        (P, out_dim)))
    return [
        {
            "x_sh": np.ascontiguousarray(x[c * b_sh:(c + 1) * b_sh]),
            "w_sl": np.ascontiguousarray(W[c * w_sl:(c + 1) * w_sl]),
            "b2_rep": b2_rep,
        }
        for c in range(ncores)
    ]


def kernel(x, W, b):
    from concourse.bass_utils import run_bass_kernel_spmd

    x = np.asarray(x, np.float32)
    W = np.asarray(W, np.float32)
    b = np.asarray(b, np.float32)
    nc = _get_nc()
    in_maps = make_in_maps(x, W, b)
    res = run_bass_kernel_spmd(nc, in_maps, core_ids=list(range(NCORES)))
    return np.concatenate(
        [np.asarray(res.results[c]["out_sh"]).astype(np.float32)
         for c in range(NCORES)], axis=0)


# revision 5
# speedup vs baseline: 3.0915x; 1.2686x over previous
"""BFLinear (block-floating-point quantized linear) Trainium2 kernel.

Computes: out = bf_quant(bf_quant(x) @ bf_quant(W).T + 2*b)
where bf_quant quantizes groups of 32 along the last axis to a shared
power-of-two exponent with 8 mantissa bits (values = int8 * 2^(e-7)).

Distribution over 8 NeuronCores:
  - batch dim of x sharded 8 ways (1024 rows/core)
  - W quantization split by output rows (512 rows/core); the quantized
    slab is transposed to [in, out] layout on the PE array (identity
    matmuls) while the PE is otherwise idle, then AllGathered as bf16 in
    two 256-column halves so matmuls against the first half can start
    while the second AllGather is still in flight
  - x is quantized on-chip and PE-transposed into a resident SBUF
    [in, batch] buffer - no DRAM round trip
  - matmul runs in bf16 (quantized values are exact in bf16), fp32 PSUM
    accumulation, k-innermost per PSUM bank; weight slabs stream from the
    gathered buffer as contiguous [128, 32, 256] tiles (no transposed DMA
    in the hot loop)
  - bias (x2) is added during the PSUM drain on the vector engine (fp32)
    - NOT pre-loaded into PSUM, which loses it to the has_written
    overwrite on fresh banks - then the output is quantized and stored as
    bf16 (exact for quantized values); the host casts back to fp32

Quantization math (matching jnp semantics):
  m     = max |x| over each group of 32          (DVE abs-max reduce)
  scale = 2^(floor(log2 m) - 7)                  (exponent-field bit math)
  inv   = 1/scale                                (bit math, exact)
  v     = clamp(x*inv, -128.25, 127.25)          (== post-round clip, round
                                                  is monotonic)
  r     = rne_round(v) via +C trick, C = 1.5*2^23 (scalar engine)
  q     = (r - C) * scale                        (exact in bf16)
"""

import numpy as np

# full-problem dimensions (hardcoded per harness contract)
B_FULL = 8192
IN_FULL = 4096
OUT_FULL = 4096
NCORES = 8

P = 128
SZ = 32
C_RND = float(3 * 2**22)  # 1.5*2^23: v+C stays in [2^23, 2^24) -> RNE to ints


def build_nc(b_sh=B_FULL // NCORES, in_dim=IN_FULL, out_dim=OUT_FULL,
             ncores=NCORES, for_timeline=False):
    """Build the SPMD Bass program (identical on every core; data differs)."""
    import concourse.mybir as mybir
    import concourse.tile as tile
    from concourse import bacc, masks

    F32 = mybir.dt.float32
    BF16 = mybir.dt.bfloat16
    I32 = mybir.dt.int32
    ALU = mybir.AluOpType
    AX = mybir.AxisListType
    AF = mybir.ActivationFunctionType

    w_sl = out_dim // ncores      # W rows quantized on this core (512)
    k_chunks = in_dim // P        # 128-wide contraction chunks (32)
    nbb = b_sh // P               # batch row blocks (8)
    half = w_sl // 2              # o-columns per AG half (256)
    g_half = half // SZ           # quant groups per drain tile (8)

    nc = bacc.Bacc("TRN2", target_bir_lowering=False, debug=False,
                   num_devices=ncores)

    x_sh = nc.dram_tensor("x_sh", [b_sh, in_dim], F32, kind="ExternalInput")
    w_sl_t = nc.dram_tensor("w_sl", [w_sl, in_dim], F32, kind="ExternalInput")
    b2_rep = nc.dram_tensor("b2_rep", [P, out_dim], F32, kind="ExternalInput")
    out_sh = nc.dram_tensor("out_sh", [b_sh, out_dim], BF16,
                            kind="ExternalOutput")

    wqT_lo = nc.dram_tensor("wqT_lo", [in_dim, half], BF16)
    wqT_hi = nc.dram_tensor("wqT_hi", [in_dim, half], BF16)
    ag_lo = nc.dram_tensor("ag_lo", [ncores * in_dim, half], BF16,
                           addr_space="Shared")
    ag_hi = nc.dram_tensor("ag_hi", [ncores * in_dim, half], BF16,
                           addr_space="Shared")

    with tile.TileContext(nc) as tc:
        from contextlib import ExitStack
        with ExitStack() as ctx:
            xpool = ctx.enter_context(tc.tile_pool(name="xpool", bufs=2))
            qpool = ctx.enter_context(tc.tile_pool(name="qpool", bufs=2))
            spool = ctx.enter_context(tc.tile_pool(name="spool", bufs=2))
            wpool = ctx.enter_context(tc.tile_pool(name="wpool", bufs=2))
            bpool = ctx.enter_context(tc.tile_pool(name="bpool", bufs=2))
            dpool = ctx.enter_context(tc.tile_pool(name="dpool", bufs=3))
            opool = ctx.enter_context(tc.tile_pool(name="opool", bufs=3))
            dsp = ctx.enter_context(tc.tile_pool(name="dsp", bufs=3))
            pmm = ctx.enter_context(
                tc.tile_pool(name="pmm", bufs=5, space="PSUM"))
            ptp = ctx.enter_context(
                tc.tile_pool(name="ptp", bufs=3, space="PSUM"))

            singles = ctx.enter_context(tc.tile_pool(name="singles", bufs=1))
            ident = singles.tile([P, P], BF16, tag="ident")
            masks.make_identity(nc, ident[:])

            # resident transposed operands
            xqT = singles.tile([P, k_chunks, b_sh], BF16, tag="xqT")
            wqt = singles.tile([P, k_chunks, w_sl], BF16, tag="wqt")

            def quant(t, width, q_out, sp, tagp):
                """Quantize an SBUF-resident [P, width] f32 tile (t is
                clobbered) into q_out (any dtype; bf16 is exact)."""
                g = width // SZ
                t3 = t.rearrange("p (g s) -> p g s", s=SZ)
                m = sp.tile([P, g], F32, tag=f"{tagp}_m")
                nc.vector.tensor_reduce(m[:], t3, axis=AX.X, op=ALU.max,
                                        apply_absolute_value=True)
                scale = sp.tile([P, g], F32, tag=f"{tagp}_scale")
                # scale_bits = (m_bits & 0x7F800000) - (7 << 23)
                nc.vector.tensor_scalar(
                    scale[:].bitcast(I32), m[:].bitcast(I32),
                    0x7F800000, None, op0=ALU.bitwise_and)
                nc.vector.tensor_scalar(
                    scale[:].bitcast(I32), scale[:].bitcast(I32),
                    7 << 23, None, op0=ALU.subtract)
                inv = sp.tile([P, g], F32, tag=f"{tagp}_inv")
                # inv_bits = (254<<23) - scale_bits
                #          = (scale_bits ^ -1) + ((254<<23)+1)
                nc.vector.tensor_scalar(
                    inv[:].bitcast(I32), scale[:].bitcast(I32),
                    -1, None, op0=ALU.bitwise_xor)
                nc.vector.tensor_scalar(
                    inv[:].bitcast(I32), inv[:].bitcast(I32),
                    (254 << 23) + 1, None, op0=ALU.add)
                # v = x * inv (exact power-of-two scaling)
                nc.vector.tensor_tensor(
                    t3, t3, inv[:, :, None].to_broadcast([P, g, SZ]),
                    ALU.mult)
                # pre-round clamp; == post-round clip (round is monotonic)
                nc.vector.tensor_scalar(
                    t, t, -128.25, 127.25, op0=ALU.max, op1=ALU.min)
                # +C forces RNE-to-integer on the scalar engine
                nc.scalar.activation(t, t, AF.Copy, bias=C_RND, scale=1.0)
                # q = (r - C) * scale, fused subtract+scale
                nc.vector.scalar_tensor_tensor(
                    q_out.rearrange("p (g s) -> p g s", s=SZ),
                    t3, C_RND,
                    scale[:, :, None].to_broadcast([P, g, SZ]),
                    op0=ALU.subtract, op1=ALU.mult)

            def pe_transpose_into(src_bf16, dest, col_base):
                """PE-transpose [P, in_dim] bf16 src into dest[:, k, col_base:
                col_base+P] for every k chunk (dest is a [P, k_chunks, *]
                SBUF tile)."""
                for k in range(k_chunks):
                    pst = ptp.tile([P, P], BF16, tag="pst",
                                   padded_shape=[P, 1024])
                    nc.tensor.matmul(pst[:], lhsT=src_bf16[:, k * P:(k + 1) * P],
                                     rhs=ident[:], is_transpose=True,
                                     skip_group_check=True)
                    nc.scalar.copy(dest[:, k, col_base:col_base + P], pst[:])

            # ---- W slice: quantize + PE-transpose into wqt ------------------
            for r in range(w_sl // P):
                wt = xpool.tile([P, in_dim], F32, tag="ld")
                nc.sync.dma_start(wt[:], w_sl_t.ap()[r * P:(r + 1) * P, :])
                wq = qpool.tile([P, in_dim], BF16, tag="q")
                quant(wt[:], in_dim, wq[:], spool, "q")
                pe_transpose_into(wq[:], wqt, r * P)

            # ---- store transposed slab halves + AllGather them -------------
            nc.gpsimd.dma_start(
                wqT_lo.ap().rearrange("(k p) o -> p k o", p=P),
                wqt[:, :, 0:half])
            nc.gpsimd.dma_start(
                wqT_hi.ap().rearrange("(k p) o -> p k o", p=P),
                wqt[:, :, half:w_sl])
            if for_timeline:
                nc.sync.dma_start(ag_lo.ap()[0:in_dim, :], wqT_lo.ap())
                nc.sync.dma_start(ag_hi.ap()[0:in_dim, :], wqT_hi.ap())
            else:
                nc.gpsimd.collective_compute(
                    "AllGather", ALU.bypass,
                    replica_groups=[list(range(ncores))],
                    ins=[wqT_lo.ap().opt()], outs=[ag_lo.ap().opt()])
                nc.gpsimd.collective_compute(
                    "AllGather", ALU.bypass,
                    replica_groups=[list(range(ncores))],
                    ins=[wqT_hi.ap().opt()], outs=[ag_hi.ap().opt()])

            # ---- x: quantize + PE-transpose into resident xqT --------------
            for bb in range(nbb):
                xt = xpool.tile([P, in_dim], F32, tag="ld")
                nc.sync.dma_start(xt[:], x_sh.ap()[bb * P:(bb + 1) * P, :])
                xq = qpool.tile([P, in_dim], BF16, tag="q")
                quant(xt[:], in_dim, xq[:], spool, "q")
                pe_transpose_into(xq[:], xqT, bb * P)

            # ---- matmul waves: 8 lo-half units, then 8 hi-half units -------
            for h, ag in ((0, ag_lo), (1, ag_hi)):
                for j in range(ncores):
                    col = j * w_sl + h * half
                    slab = wpool.tile([P, k_chunks, half], BF16, tag="slab")
                    nc.scalar.dma_start(
                        slab[:],
                        ag.ap()[j * in_dim:(j + 1) * in_dim, :]
                        .rearrange("(k p) o -> p k o", p=P))
                    b2s = bpool.tile([P, half], F32, tag="b2s")
                    nc.scalar.dma_start(b2s[:], b2_rep.ap()[:, col:col + half])
                    for bb in range(nbb):
                        ps = pmm.tile([P, half], F32, tag="ps",
                                      padded_shape=[P, 512])
                        for k in range(k_chunks):
                            nc.tensor.matmul(
                                ps[:],
                                lhsT=xqT[:, k, bb * P:(bb + 1) * P],
                                rhs=slab[:, k, :],
                                start=(k == 0), stop=(k == k_chunks - 1),
                                skip_group_check=True)
                        s = dpool.tile([P, half], F32, tag="s")
                        # bias (x2, baked into b2_rep host-side) added during
                        # the PSUM drain - exact fp32
                        nc.vector.tensor_tensor(s[:], ps[:], b2s[:], ALU.add)
                        oq = opool.tile([P, half], BF16, tag="oq")
                        quant(s[:], half, oq[:], dsp, "d")
                        nc.gpsimd.dma_start(
                            out_sh.ap()[bb * P:(bb + 1) * P, col:col + half],
                            oq[:])

    nc.compile()
    return nc


_NC_CACHE = {}


def _get_nc(key=(B_FULL // NCORES, IN_FULL, OUT_FULL, NCORES)):
    if key not in _NC_CACHE:
        _NC_CACHE[key] = build_nc(*key)
    return _NC_CACHE[key]


def make_in_maps(x, W, b, ncores=NCORES):
    b_sh = x.shape[0] // ncores
    w_sl = W.shape[0] // ncores
    out_dim = W.shape[0]
    b2_rep = np.ascontiguousarray(
        np.broadcast_to((2.0 * np.asarray(b, np.float32)).reshape(1, out_dim),
                        (P, out_dim)))
    return [
        {
            "x_sh": np.ascontiguousarray(x[c * b_sh:(c + 1) * b_sh]),
            "w_sl": np.ascontiguousarray(W[c * w_sl:(c + 1) * w_sl]),
            "b2_rep": b2_rep,
        }
        for c in range(ncores)
    ]


def kernel(x, W, b):
    from concourse.bass_utils import run_bass_kernel_spmd

    x = np.asarray(x, np.float32)
    W = np.asarray(W, np.float32)
    b = np.asarray(b, np.float32)
    nc = _get_nc()
    in_maps = make_in_maps(x, W, b)
    res = run_bass_kernel_spmd(nc, in_maps, core_ids=list(range(NCORES)))
    return np.concatenate(
        [np.asarray(res.results[c]["out_sh"]).astype(np.float32)
         for c in range(NCORES)], axis=0)
